# revision 57
# baseline (speedup 1.0000x reference)
"""Trainium2 Bass kernel for nn_CrossNetwork (DCN-v1 cross network).

Math: reference computes x_{i+1} = input * (x_i . w_i) + x_i + b_i, L=6 layers.
Writing x_i = input * c_i + B_i with B_i = sum_{j<i} b_j (a constant row
vector) and c_i a per-row scalar, the recursion collapses to
    u_i    = input . w_i                     (per row, one tall-skinny matmul)
    beta_i = B_i . w_i                       (host-computed constants)
    c_{i+1} = c_i * (1 + u_i) + beta_i ; c_0 = 1
    out    = input * c_L + B_L
For the b == 0 case this is out = input * prod_i(1 + u_i).

Device work per core (2048 rows): load x once, PE-transpose 128x128 blocks
(f32r, 1.5 cyc/row), f32r matmul against W^T accumulating U^T[6, rows]
(1 cyc/row), DVE product-reduce to c, DVE per-partition-scalar multiply,
store. HBM traffic 16MB/core => memory-bound, roofline ~47us at 358GB/s.

Perf history: 70.9us (first session) -> 52.2us (v1 shipped) -> 45.5us
(v2, this session). Measured HW facts driving v2 (see also the session
memory trn2-crossnetwork-findings):
  - Reads and writes do NOT overlap on this HW path (dma_nodep split-ring
    test = serial sum); aggregate ~310-320 GB/s/core. The kernel's exact
    DMA pattern with no compute (MODE=mix) runs 40.5us/iter = the floor.
  - v1 was TENSOR-ENGINE bound, not DMA bound: f32r 128x128 transposes
    at 1.5 cyc/row + ~77ns/instr fixed cost x128, plus an 8.4us delayed
    PE start behind the first 2MB load piece (fullnoload diag = 40.6,
    fullnostore = 49.1).
v2 pipeline (default, K_V2=1): 16 per-chunk 512KB loads (PE starts after
the first 512KB), DVE pre-cast f32->bf16 (transposes drop to 1 cyc/row,
DVE scale gets 16-bit 2x), chunk-major transpose emission in two 4-k
passes into bf16 PSUM tiles (4 pxt bufs + ut/uj = exactly 8 PSUM banks),
ACT PSUM->SBUF copies, bf16 matmuls (N=512), c = prod(1+u_i) mult-reduce,
bf16 y sub-stores every 2 chunks. Knobs that measured WORSE: ISSUE_LAZY,
PIECES=8/GRP=2 in v1, KSP=2, PU_BUFS=1, PXT_BUFS>4 (PSUM overflow),
CAST=act/mix, SUB_EVERY=1/4/8, staggered For_i reset (codegen failure),
SWDGE cast-on-load (walrus can't codegen SWDGE DMA inside For_i).
"""

import os

import numpy as np

import concourse.bass as bass
import concourse.mybir as mybir
import concourse.tile as tile
from concourse.bass_utils import run_bass_kernel_spmd
from concourse.masks import make_identity
from concourse.vector_clock import ScopedClock

F32 = mybir.dt.float32
BF16 = mybir.dt.bfloat16

B, D, L = 16384, 1024, 6
NCORES = 8
R = B // NCORES  # rows per core
P = 128
NCH = R // P  # chunks of 128 rows per core
KB = D // P  # 128-wide k blocks
# chunks per psum accumulation group (matmul N = GRP*128; 4 => 512, the
# fp32 PSUM bank cap)
GRP = int(os.environ.get("K_GRP", "4"))
NG = NCH // GRP

# --- tuning knobs (env-overridable for A/B; defaults are the shipped config)
MM_BF16 = os.environ.get("K_MM_BF16", "1") == "1"
PIECES = int(os.environ.get("K_PIECES", "4"))  # DMA pieces per direction
LOAD_ENG = os.environ.get("K_LOAD_ENG", "sync")
STORE_ENG = os.environ.get("K_STORE_ENG", "sync")
# full | dma | load | load2 | store | dma_nodep
MODE = os.environ.get("K_MODE", "full")
# store granularity in units of GRP-chunk groups (1 => 2MB stores)
STORE_GRP = int(os.environ.get("K_STORE_GRP", "1"))
# alternate HWDGE rings per transfer: none | stores | loads | both
RING_SPLIT = os.environ.get("K_RING_SPLIT", "none")
# send odd-chunk row-scales to ACT instead of DVE
SCALE_SPLIT = os.environ.get("K_SCALE_SPLIT", "0") == "1"
# issue load piece h+2 after store h instead of all loads upfront, so the
# HWDGE ring interleaves reads and writes (requires GPC == GRP)
ISSUE_LAZY = os.environ.get("K_ISSUE_LAZY", "0") == "1"
# store y as bf16 (half the write traffic), upcast host-side; output
# quantization ~2e-3 rel err, inside the 2e-2 gate
OUT_BF16 = os.environ.get("K_OUT_BF16", "1") == "1"
# bf16 path only: store every 2 chunks (512KB) as soon as scaled
STORE_SUB = os.environ.get("K_STORE_SUB", "1") == "1"
# chunks per sub-store (2 => 512KB pairs, 1 => 256KB per chunk)
SUB_EVERY = int(os.environ.get("K_SUB_EVERY", "2"))
# x lands in SBUF as bf16 via SWDGE cast-on-load (gpsimd dma); PE
# transposes then run at 1 cyc/row (vs 1.5 f32r) and the DVE scale gets
# 2x 16-bit throughput.  NOTE: this walrus build cannot codegen SWDGE
# DMA inside a For_i loop ("ISA wrong length"), so only usable unlooped.
X_BF16 = os.environ.get("K_X_BF16", "0") == "1"
# software-pipeline lead: how many (g, k) steps the transpose+copy runs
# ahead of the matmul consuming it
LEAD = int(os.environ.get("K_LEAD", "1"))
# v2 body: per-chunk 512KB loads (early PE start), DVE pre-cast of x to
# bf16 (PE transposes 1.5->1.0 cyc/row), chunk-major transpose emission
# (PE never waits on a chunk beyond the one being transposed), bf16 PSUM
# transpose tiles, bf16 2x DVE scale
V2 = os.environ.get("K_V2", "1") == "1"
# engine for the f32->bf16 pre-cast: dve | act | mix (alternate)
CAST_ENG = os.environ.get("K_CAST", "dve")
# f32 x-chunk staging buffers (cast is the only consumer)
XF_BUFS = int(os.environ.get("K_XF_BUFS", "6"))
PXT_BUFS = int(os.environ.get("K_PXT_BUFS", "4"))
XB_BUFS = int(os.environ.get("K_XB_BUFS", "8"))
# v2 diagnostics: decouple stores / loads from the compute chain
V2NOSTORE = os.environ.get("K_V2NOSTORE", "0") == "1"
V2NOLOAD = os.environ.get("K_V2NOLOAD", "0") == "1"
# v2 k's per transpose pass (live pxt tiles per pass)
KSP = int(os.environ.get("K_KSP", "4"))
# For_i staggered semaphore reset (skips the full all-engine barrier).
# NOTE: fails walrus codegen (no_semaphore_value_conflict) — leave off.
STAGGER = os.environ.get("K_STAGGER", "0") == "1"
PU_BUFS = int(os.environ.get("K_PU_BUFS", "2"))
GPC = NCH // PIECES  # chunks per DMA piece
assert NCH % PIECES == 0 and GPC % GRP == 0
assert (GPC // GRP) % STORE_GRP == 0  # stores must not span piece tiles


def _patch_tile_drain():
    """This walrus build rejects >1 sem wait on a CTRL (Drain) instruction.

    Tile's kernel-tail drain waits on every sem domain at once; split it into
    chained single-wait drains.
    """
    if getattr(tile.TileContext, "_drain_patched", False):
        return

    def _drain_and_barrier(self, tick_clock, wait_clock):
        gc = tick_clock.global_clock
        entries = [(proc, t) for proc, t in enumerate(gc) if t > 0]
        if not entries:
            self.nc.sync.drain()
        for proc, t in entries:
            sub = ScopedClock()
            sub.require_at_least(None, proc, t)
            drain_inst = self.nc.sync.drain()
            wait_clock.add_sem_waits(drain_inst.ins, sub)

        self.nc.all_engine_barrier()
        assert self.sems is not None
        popped = self.nc._tile_sem_poison_stack.pop()
        assert popped is self._sem_poison
        self.nc.clear_and_free_semaphores(list(self.sems.allocated().values()))

    tile.TileContext._drain_and_barrier = _drain_and_barrier
    tile.TileContext._drain_patched = True


def _build(with_bias: bool, loop_n: int = 1):
    nc = bass.Bass("TRN2")
    x_d = nc.dram_tensor("x", [R, D], F32, kind="ExternalInput")
    wt_d = nc.dram_tensor("wt", [D, L], F32, kind="ExternalInput")
    if with_bias:
        bl_d = nc.dram_tensor("bl", [1, D], F32, kind="ExternalInput")
        beta_d = nc.dram_tensor("beta", [1, L], F32, kind="ExternalInput")
    out_bf = OUT_BF16 and not with_bias and MODE in (
        "full", "mix", "fullnostore", "fullnoload"
    )
    y_d = nc.dram_tensor(
        "y", [R, D], BF16 if out_bf else F32, kind="ExternalOutput"
    )

    xv = x_d.rearrange("(p n) d -> p n d", p=P)  # [128, NCH, D]
    yv = y_d.rearrange("(p n) d -> p n d", p=P)
    wtv = wt_d.rearrange("(k p) s -> p k s", p=P)  # [128, KB, L]

    use_v2 = V2 and not with_bias and MODE == "full" and out_bf
    with tile.TileContext(nc) as tc:
        nwin = NCH // (STORE_GRP * GRP)
        with (
            tc.tile_pool(name="consts", bufs=1) as consts,
            tc.tile_pool(name="xch", bufs=XF_BUFS if use_v2 else PIECES)
            as xpool,
            tc.tile_pool(name="xb", bufs=XB_BUFS) as xbpool,
            tc.tile_pool(name="xt", bufs=4) as xtpool,
            tc.tile_pool(name="small", bufs=2 * GRP) as small,
            tc.tile_pool(name="ybf", bufs=3 if use_v2 else min(nwin, 4))
            as ypool,
            tc.tile_pool(
                name="pxt", bufs=PXT_BUFS if use_v2 else 4, space="PSUM"
            ) as pxt,
            tc.tile_pool(
                name="pu", bufs=PU_BUFS if use_v2 else 2, space="PSUM"
            ) as pu,
        ):
            ident = consts.tile([P, P], F32)
            make_identity(nc, ident)
            identx = ident
            if (X_BF16 or use_v2) and not with_bias:
                identx = consts.tile([P, P], BF16)
                make_identity(nc, identx)
            src_sb = None
            if MODE in ("store", "dma_nodep"):
                src_sb = []
                for h in range(PIECES):
                    t = consts.tile([P, GPC, D], F32, name=f"src{h}")
                    nc.vector.memset(t, 1.0)
                    src_sb.append(t)
            elif MODE in ("mix", "fullnostore"):
                t = consts.tile(
                    [P, SUB_EVERY, D], BF16 if out_bf else F32, name="srcy"
                )
                nc.vector.memset(t, 1.0)
                src_sb = [t]
            elif MODE == "fullnoload":
                # compute reads preset tiles; loads still stream (nodep)
                src_sb = []
                for h in range(PIECES):
                    t = consts.tile([P, GPC, D], F32, name=f"xc{h}")
                    nc.vector.memset(t, 0.25)
                    src_sb.append(t)
            xconst = yconst = None
            if use_v2 and V2NOLOAD:
                xconst = consts.tile([P, D], F32, name="xconst")
                nc.vector.memset(xconst, 0.25)
            if use_v2 and V2NOSTORE:
                yconst = consts.tile([P, SUB_EVERY, D], BF16, name="yconst")
                nc.vector.memset(yconst, 1.0)
            ident6 = consts.tile([L, L], F32)
            make_identity(nc, ident6)
            wt_sb = consts.tile([P, KB, L], F32)
            nc.sync.dma_start(out=wt_sb, in_=wtv)
            if MM_BF16:
                wt_bf = consts.tile([P, KB, L], BF16)
                nc.scalar.copy(wt_bf, wt_sb)
                wt_sb = wt_bf
            bl_sb = beta_sb = None
            if with_bias:
                bl_sb = consts.tile([P, D], F32)
                nc.sync.dma_start(
                    out=bl_sb,
                    in_=bass.AP(tensor=bl_d, offset=0, ap=[[0, P], [1, D]]),
                )
                beta_sb = consts.tile([P, L], F32)
                nc.sync.dma_start(
                    out=beta_sb,
                    in_=bass.AP(tensor=beta_d, offset=0, ap=[[0, P], [1, L]]),
                )

            import contextlib
            loop_cm = (
                tc.For_i(0, loop_n, 1, staggered_reset=STAGGER)
                if loop_n > 1
                else contextlib.nullcontext()
            )
            with loop_cm:
                if use_v2:
                    _body_v2(nc, tc, xpool, xbpool, xtpool, small, ypool,
                             pxt, pu, identx, ident6, wt_sb, xv, yv,
                             xconst, yconst)
                else:
                    _body(nc, tc, xpool, xtpool, small, ypool, pxt, pu,
                          ident, ident6, wt_sb, bl_sb, beta_sb, xv, yv,
                          with_bias, src_sb, out_bf, identx)
    return nc


def _body_v2(nc, tc, xpool, xbpool, xtpool, small, ypool, pxt, pu,
             identx, ident6, wt_sb, xv, yv, xconst=None, yconst=None):
    """PE-bound-optimized pipeline, b == 0 only, bf16 output.

    Per-chunk loads -> pre-cast to bf16 -> chunk-major PE transposes into
    per-(g,k) bf16 PSUM tiles -> ACT copy to SBUF -> bf16 matmul -> group
    tail (1+u product, row scale) -> 512KB sub-stores.
    """
    xv1 = xv  # [P, NCH, D]

    # per-chunk f32 staging: cast is the only consumer, small rotation
    x_sb = {}

    def load_chunk(n):
        t = xpool.tile([P, D], F32, tag="xf")
        nc.sync.dma_start(out=t, in_=xv1[:, n, :])
        x_sb[n] = t

    for n in range(NCH):
        load_chunk(n)

    cast_engs = {
        "dve": [nc.vector.tensor_copy],
        "act": [nc.scalar.copy],
        "mix": [nc.vector.tensor_copy, nc.scalar.copy],
    }[CAST_ENG]
    xb = {}

    def emit_cast(n):
        t = xbpool.tile([P, D], BF16, tag="xb")
        src = x_sb.pop(n)
        cast_engs[n % len(cast_engs)](t, xconst if V2NOLOAD else src)
        xb[n] = t

    ywin = {}

    def emit_group(g):
        ut = pu.tile([L, GRP * P], F32, tag="u", name=f"ut{g}")
        for k0 in range(0, KB, KSP):
            pxt_t = {}
            for j in range(GRP):
                n = g * GRP + j
                if k0 == 0:
                    emit_cast(n)
                for k in range(k0, k0 + KSP):
                    if j == 0:
                        pxt_t[k] = pxt.tile(
                            [P, GRP * P], BF16, tag="pxt",
                            name=f"pxt{g}_{k}",
                        )
                    nc.tensor.transpose(
                        pxt_t[k][:, j * P : (j + 1) * P],
                        xb[n][:, k * P : (k + 1) * P],
                        identx,
                    )
            for k in range(k0, k0 + KSP):
                xt = xtpool.tile([P, GRP * P], BF16, tag="xt")
                nc.scalar.copy(xt, pxt_t[k])
                nc.tensor.matmul(
                    ut[:], wt_sb[:, k, :], xt[:],
                    start=(k == 0), stop=(k == KB - 1),
                )
        # tail: c = prod(1 + u_i), y = x * c
        u1t = xtpool.tile([L, GRP * P], F32, tag="u1t")
        nc.vector.tensor_scalar_add(u1t, ut, 1.0)
        uj = pu.tile([P, GRP, L], F32, tag="uj", name=f"uj{g}")
        for j in range(GRP):
            nc.tensor.transpose(
                uj[:, j, :], u1t[:, j * P : (j + 1) * P], ident6
            )
        yw = ypool.tile([P, GRP, D], BF16, tag="yw", name=f"yw{g}")
        ywin[g] = yw
        # last group: per-chunk 256KB stores so the final store enters the
        # ring as early as possible (shorter drain tail)
        sub = SUB_EVERY
        if g == NCH // GRP - 1 and not V2NOSTORE:
            sub = 1
        for j in range(GRP):
            n = g * GRP + j
            c_t = small.tile([P, 1], F32, tag="c")
            nc.vector.tensor_reduce(
                c_t,
                uj[:, j, :],
                axis=mybir.AxisListType.X,
                op=mybir.AluOpType.mult,
            )
            nc.vector.tensor_scalar_mul(yw[:, j, :], xb.pop(n), c_t)
            if (j + 1) % sub == 0:
                c0 = j + 1 - sub
                nc.sync.dma_start(
                    out=yv[:, n + 1 - sub : n + 1, :],
                    in_=(
                        yconst[:]
                        if V2NOSTORE
                        else yw[:, c0 : c0 + sub, :]
                    ),
                )
        ywin.pop(g)

    for g in range(NCH // GRP):
        emit_group(g)


def _body(nc, tc, xpool, xtpool, small, ypool, pxt, pu, ident, ident6,
          wt_sb, bl_sb, beta_sb, xv, yv, with_bias, src_sb=None,
          out_bf=False, identx=None):
    load_eng = getattr(nc, LOAD_ENG)
    store_eng = getattr(nc, STORE_ENG)
    x_bf = X_BF16 and not with_bias
    if identx is None or not x_bf:
        identx = ident
    x_dt = BF16 if x_bf else F32

    if MODE in ("store", "dma_nodep"):
        # stores sourced from preset const tiles — no dependency coupling
        if MODE == "dma_nodep":
            for h in range(PIECES):
                t = xpool.tile([P, GPC, D], F32, tag="xch")
                load_eng.dma_start(
                    out=t, in_=xv[:, h * GPC : (h + 1) * GPC, :]
                )
        for h in range(PIECES):
            store_eng.dma_start(
                out=yv[:, h * GPC : (h + 1) * GPC, :], in_=src_sb[h]
            )
            if MODE == "store":
                store_eng.dma_start(
                    out=yv[:, h * GPC : (h + 1) * GPC, :], in_=src_sb[h]
                )
        return

    if MODE == "mix":
        # the full kernel's exact DMA pattern, no compute, no deps:
        # PIECES f32 loads + per-SUB_EVERY bf16 stores (out_bf build only)
        for h in range(PIECES):
            t = xpool.tile([P, GPC, D], x_dt, tag="xch")
            load_eng.dma_start(out=t, in_=xv[:, h * GPC : (h + 1) * GPC, :])
        yconst = src_sb[0]  # [P, SUB_EVERY, D] bf16 const tile
        for n0 in range(0, NCH, SUB_EVERY):
            store_eng.dma_start(
                out=yv[:, n0 : n0 + SUB_EVERY, :], in_=yconst
            )
        return

    # big-piece loads: PIECES tiles of [128, GPC, D]
    lazy = ISSUE_LAZY and GPC == GRP and MODE == "full"
    x_sb = [None] * PIECES

    def load_piece(h):
        t = xpool.tile([P, GPC, D], x_dt, tag="xch")
        eng = load_eng
        if MODE == "load2" or RING_SPLIT in ("loads", "both"):
            eng = (nc.sync, nc.scalar)[h % 2]
        if x_bf:
            eng = nc.gpsimd  # only SWDGE can cast f32->bf16 during DMA
        eng.dma_start(out=t, in_=xv[:, h * GPC : (h + 1) * GPC, :])
        x_sb[h] = t

    for h in range(2 if lazy else PIECES):
        load_piece(h)

    if MODE in ("load", "load2"):
        # 16MB of pure reads (two passes over the 8MB input)
        for h in range(PIECES):
            eng = load_eng if MODE != "load2" else (nc.sync, nc.scalar)[h % 2]
            if x_bf:
                eng = nc.gpsimd
            eng.dma_start(out=x_sb[h], in_=xv[:, h * GPC : (h + 1) * GPC, :])
        return

    if MODE == "dma":
        for h in range(PIECES):
            store_eng.dma_start(
                out=yv[:, h * GPC : (h + 1) * GPC, :], in_=x_sb[h]
            )
        return

    def xch(n):
        # chunk n as a [128, D] view into its piece tile
        src = src_sb if MODE == "fullnoload" else x_sb
        return src[n // GPC][:, n % GPC, :]

    # software pipeline: emit transposes for (g,k) one step ahead of the
    # matmul consuming (g,k-1)'s copied tile, so PE never waits on ACT.
    flat = [(g, k) for g in range(NG) for k in range(KB)]
    xt_tiles = {}
    xt_dt = BF16 if MM_BF16 else F32

    def emit_transpose_copy(g, k):
        # PE transpose requires out dtype == in dtype (bf16 PSUM when x_bf)
        pxt_t = pxt.tile([P, GRP * P], x_dt, tag="pxt")
        for j in range(GRP):
            src = xch(g * GRP + j)[:, k * P : (k + 1) * P]
            nc.tensor.transpose(pxt_t[:, j * P : (j + 1) * P], src, identx)
        xt_t = xtpool.tile([P, GRP * P], xt_dt, tag="xt")
        nc.scalar.copy(xt_t, pxt_t)
        xt_tiles[(g, k)] = xt_t

    ut_tiles = {}

    def emit_matmul(g, k):
        if k == 0:
            ut_tiles[g] = pu.tile([L, GRP * P], F32, tag="u", name=f"ut{g}")
        lhs = wt_sb[:, k, :]
        rhs = xt_tiles.pop((g, k))[:]
        nc.tensor.matmul(
            ut_tiles[g][:], lhs, rhs, start=(k == 0), stop=(k == KB - 1)
        )

    ywin = {}

    def emit_group_tail(g):
        # 1 + U^T on DVE while copying PSUM->SBUF, transpose [6,128] blocks
        # back to row-major [128,6], product-reduce to c, scale rows.
        w, slot = g // STORE_GRP, g % STORE_GRP
        if out_bf and slot == 0:
            ywin[w] = ypool.tile(
                [P, STORE_GRP * GRP, D], BF16, tag="ybf", name=f"yw{w}"
            )
        ut_ps = ut_tiles.pop(g)
        u1t_t = xtpool.tile([L, GRP * P], F32, tag="u1t")
        nc.vector.tensor_scalar_add(u1t_t, ut_ps, 1.0)
        uj_ps = pu.tile([P, GRP, L], F32, tag="uj", name=f"uj{g}")
        for j in range(GRP):
            nc.tensor.transpose(
                uj_ps[:, j, :], u1t_t[:, j * P : (j + 1) * P], ident6
            )
        for j in range(GRP):
            n = g * GRP + j
            if not with_bias:
                # c = prod over the 6 (1+u_i): one mult-reduce off PSUM
                c_t = small.tile([P, 1], F32, tag="c")
                nc.vector.tensor_reduce(
                    c_t,
                    uj_ps[:, j, :],
                    axis=mybir.AxisListType.X,
                    op=mybir.AluOpType.mult,
                )
                y_ap = ywin[w][:, slot * GRP + j, :] if out_bf else xch(n)
                if SCALE_SPLIT and j % 2 == 1:
                    nc.scalar.activation(
                        y_ap,
                        xch(n),
                        mybir.ActivationFunctionType.Copy,
                        scale=c_t[:],
                    )
                else:
                    nc.vector.tensor_scalar_mul(y_ap, xch(n), c_t)
                if out_bf and STORE_SUB and (j + 1) % SUB_EVERY == 0:
                    c0 = slot * GRP + j + 1 - SUB_EVERY
                    src = (
                        src_sb[0][:]
                        if MODE == "fullnostore"
                        else ywin[w][:, c0 : c0 + SUB_EVERY, :]
                    )
                    store_eng.dma_start(
                        out=yv[:, n + 1 - SUB_EVERY : n + 1, :], in_=src
                    )
            else:
                u1_t = small.tile([P, L], F32, tag="u1")
                nc.vector.tensor_copy(u1_t, uj_ps[:, j, :])
                c_t = small.tile([P, 1], F32, tag="c")
                nc.vector.memset(c_t, 1.0)
                for i in range(L):
                    # c = c * (1 + u_i) + beta_i
                    nc.vector.scalar_tensor_tensor(
                        out=c_t,
                        in0=c_t,
                        scalar=u1_t[:, i : i + 1],
                        in1=beta_sb[:, i : i + 1],
                        op0=mybir.AluOpType.mult,
                        op1=mybir.AluOpType.add,
                    )
                # out = x * c + B_L
                nc.vector.scalar_tensor_tensor(
                    out=xch(n),
                    in0=xch(n),
                    scalar=c_t,
                    in1=bl_sb,
                    op0=mybir.AluOpType.mult,
                    op1=mybir.AluOpType.add,
                )
        if out_bf and STORE_SUB:
            if (g + 1) % STORE_GRP == 0:
                w = g // STORE_GRP
                ywin.pop(w)
                # sub-stores already issued in the j loop; with GPC == GRP
                # and STORE_GRP == 1 window w maps 1:1 to load piece w, so
                # trigger the deferred load here to interleave the ring as
                # L0 L1 S0 L2 S1 L3 S2 S3
                if lazy and w + 2 < PIECES:
                    load_piece(w + 2)
            return
        # store once STORE_GRP groups are scaled
        if (g + 1) % STORE_GRP == 0:
            s = g // STORE_GRP  # store index
            n0 = (g + 1 - STORE_GRP) * GRP  # first chunk of this store
            n1 = (g + 1) * GRP
            h = n0 // GPC
            eng = store_eng
            if RING_SPLIT in ("stores", "both"):
                eng = (nc.sync, nc.scalar)[s % 2]
            src = (
                ywin.pop(s)
                if out_bf
                else x_sb[h][:, n0 - h * GPC : n1 - h * GPC, :]
            )
            eng.dma_start(out=yv[:, n0:n1, :], in_=src)
            if lazy and s + 2 < PIECES:
                load_piece(s + 2)

    # pipelined emission
    for i in range(min(LEAD, len(flat))):
        emit_transpose_copy(*flat[i])
    for i, (g, k) in enumerate(flat):
        if i + LEAD < len(flat):
            emit_transpose_copy(*flat[i + LEAD])
        emit_matmul(g, k)
        if k == KB - 1:
            emit_group_tail(g)


def _split_multi_waits(nc):
    """This walrus build allows only one sem wait on several instruction
    structs (e.g. self-loading Matmult). Move extra waits onto preceding
    same-engine NOPs; engine FIFO order makes this equivalent."""
    n = 0
    for fn in nc.m.functions:
        for bb in fn.blocks:
            out = []
            for inst in bb.instructions:
                si = inst.sync_info
                if si is not None and si.on_wait and len(si.on_wait) > 1:
                    waits = list(si.on_wait)
                    for w in waits[:-1]:
                        n += 1
                        out.append(
                            mybir.InstNoOp(
                                name=f"nopw-{n}-{inst.name}",
                                engine=inst.engine,
                                sync_info=mybir.SyncInfo(
                                    on_wait=[w], on_update=[]
                                ),
                                bass_nofuse=True,
                            )
                        )
                    inst.sync_info = mybir.SyncInfo(
                        on_wait=[waits[-1]], on_update=list(si.on_update)
                    )
                out.append(inst)
            bb.instructions = out


_CACHE = {}


def _get_nc(with_bias: bool, loop_n: int = 1):
    key = (with_bias, loop_n, MM_BF16, PIECES, LOAD_ENG, STORE_ENG, MODE,
           STORE_GRP, RING_SPLIT, SCALE_SPLIT, GRP, ISSUE_LAZY, OUT_BF16,
           STORE_SUB, SUB_EVERY, X_BF16, LEAD, V2, CAST_ENG, XF_BUFS,
           PXT_BUFS, XB_BUFS, V2NOSTORE, V2NOLOAD, KSP, STAGGER, PU_BUFS)
    if key not in _CACHE:
        _patch_tile_drain()
        nc = _build(with_bias, loop_n)
        _split_multi_waits(nc)
        _CACHE[key] = nc
    return _CACHE[key]


def kernel(input, W, b, **run_kwargs):
    input = np.ascontiguousarray(np.asarray(input, dtype=np.float32))
    W = np.asarray(W, dtype=np.float32)
    b = np.asarray(b, dtype=np.float32)
    assert input.shape == (B, D) and W.shape == (L, D) and b.shape == (L, D)

    with_bias = bool(np.any(b))
    nc = _get_nc(with_bias)

    wt = np.ascontiguousarray(W.T)  # [D, L]
    in_maps = []
    for i in range(NCORES):
        m = {"x": input[i * R : (i + 1) * R], "wt": wt}
        if with_bias:
            # B_i = sum_{j<i} b_j ; beta_i = B_i . w_i ; B_L = sum_j b_j
            Bpre = np.concatenate(
                [np.zeros((1, D), np.float32), np.cumsum(b, axis=0)[:-1]], axis=0
            )
            m["bl"] = b.sum(axis=0, dtype=np.float32).reshape(1, D)
            m["beta"] = np.einsum("ld,ld->l", Bpre, W).astype(np.float32).reshape(1, L)
        in_maps.append(m)

    res = run_bass_kernel_spmd(
        nc, in_maps, core_ids=list(range(NCORES)), **run_kwargs
    )
    out = np.concatenate(
        [
            np.asarray(res.results[i]["y"]).astype(np.float32, copy=False)
            for i in range(NCORES)
        ],
        axis=0,
    )
    if run_kwargs:
        return out, res
    return out

